# revision 1
# baseline (speedup 1.0000x reference)
"""Trainium2 Bass kernel for nn_DMHA_3255585210402 (retrieval_knn DMHA).

Key algebraic fact: TOPK == NVK == 4, so jax.lax.top_k over the size-4 v_keys
axis selects *all* entries; the gather+sum over (DVH, TOPK) therefore reduces
to a constant vector c = 2 * v_embed[0:4].sum(0), and the whole
compute_value_states branch collapses to  v = x * c  (verified: 1.4e-7 rel).

So the module is a causal MHA layer (B=2, H=16, T=2048, HD=128, D=2048) with
elementwise-scaled V.  Sharding: 8 cores = 2 batches x 4 head-groups.  Each
core computes, for its batch b and 4 heads:
  qT/kT projections (transposed layout, feature-on-partition),
  causal softmax attention in transposed score layout (sT[tk, tq]),
  the partial output projection  outT_g = Wo[:, gsl]-slice.T @ oT.
Host sums the 4 head-group partials per batch and adds bo.

The c scale rides the per-partition scalar of the normalize multiply
(o = c[p] * (x_g.T @ w) * recip[tq]), so V is never materialized.
All matmuls run as float32r; softmax denominators use the ones-column
matmul for the partition reduction and reciprocal_approx_fast + a
DMA row-broadcast so the PE never waits on the normalization chain.
"""

import math

import numpy as np

import concourse.bass as bass
import concourse.mybir as mybir
import concourse.tile as tile
from concourse import bacc
from concourse.bass_utils import run_bass_kernel_spmd

B, T, D = 2, 2048, 2048
H, HD = 16, 128
G = 4              # head-groups (cores per batch)
GH = H // G        # heads per core
GF = GH * HD       # projected features per core (512)
NCORES = 8
P = 128            # partitions
TQ = 512           # tq chunk width (psum bank / fp32 moving max)
F32 = mybir.dt.float32
F32R = mybir.dt.float32r

DK = D // P        # 16 contraction chunks for projections
NTQ = T // TQ      # 4 query chunks
NTK = T // P       # 16 key chunks


def _body(tc, xT, xg, wqT, wkT, woT, cT, bqT, bkT, ones, out):
    nc = tc.nc
    rsqrt_hd = 1.0 / math.sqrt(HD)
    mult = mybir.AluOpType.mult

    with (
        tc.tile_pool(name="const", bufs=1) as const,
        tc.tile_pool(name="res1", bufs=1) as res1,
    ):
        # preload the gpsimd library that partition_broadcast needs so the
        # ~11us library DMA happens during phase A, not at first use
        from concourse import library_config
        with tc.high_priority():
            nc.gpsimd.load_library(library_config.attn)
        qT_sb = res1.tile([P, GH, T], F32R)   # q, transposed per head
        kT_sb = res1.tile([P, GH, T], F32R)

        # --- phase A: q/k projections, transposed layout ---
        with (
            tc.tile_pool(name="wqk", bufs=1) as wqk,
            tc.tile_pool(name="xt", bufs=20) as xtp,
            tc.tile_pool(name="psA", bufs=8, space="PSUM") as psA,
        ):
            wq_sb = wqk.tile([P, DK, GF], F32R)
            wk_sb = wqk.tile([P, DK, GF], F32R)
            wqr = wqT.rearrange("(n p) f -> p n f", p=P)
            wkr = wkT.rearrange("(n p) f -> p n f", p=P)
            xts0 = []
            for dk in range(DK):
                nc.sync.dma_start(out=wq_sb[:, dk, :], in_=wqr[:, dk, :])
                nc.sync.dma_start(out=wk_sb[:, dk, :], in_=wkr[:, dk, :])
                xt0 = xtp.tile([P, TQ], F32R, name="xt")
                nc.sync.dma_start(
                    out=xt0, in_=xT[dk * P : (dk + 1) * P, 0:TQ]
                )
                xts0.append(xt0)

            # small constants (needed from the first psum copy onward)
            ones_sb = const.tile([P, P], F32R)
            nc.sync.dma_start(out=ones_sb, in_=ones)
            bq_sb = const.tile([HD, GH], F32)
            nc.sync.dma_start(out=bq_sb, in_=bqT)
            bk_sb = const.tile([HD, GH], F32)
            nc.sync.dma_start(out=bk_sb, in_=bkT)
            cT_sb = const.tile([HD, GH], F32)
            nc.sync.dma_start(out=cT_sb, in_=cT)

            for tci in range(NTQ):
                tsl = slice(tci * TQ, (tci + 1) * TQ)
                ps = [
                    psA.tile([P, TQ], F32, name="psA_t", tag="psA_t")
                    for _ in range(2 * GH)
                ]
                for dk in range(DK):
                    if tci == 0:
                        xt = xts0[dk]
                    else:
                        xt = xtp.tile([P, TQ], F32R, name="xt")
                        nc.sync.dma_start(
                            out=xt, in_=xT[dk * P : (dk + 1) * P, tsl]
                        )
                    for w, w_sb in enumerate((wq_sb, wk_sb)):
                        for h in range(GH):
                            nc.tensor.matmul(
                                ps[w * GH + h],
                                w_sb[:, dk, h * HD : (h + 1) * HD],
                                xt,
                                start=(dk == 0),
                                stop=(dk == DK - 1),
                            )
                for w, dstT, bias in ((0, qT_sb, bq_sb), (1, kT_sb, bk_sb)):
                    for h in range(GH):
                        nc.scalar.activation(
                            dstT[:, h, tsl],
                            ps[w * GH + h],
                            mybir.ActivationFunctionType.Identity,
                            bias=bias[:, h : h + 1],
                        )

        # --- phases B+C interleaved over query chunks ---
        with (
            tc.tile_pool(name="res2", bufs=1) as res2,
            tc.tile_pool(name="wt", bufs=6) as wtp,
            tc.tile_pool(name="pr", bufs=3) as prp,
            tc.tile_pool(name="small", bufs=4) as smp,
            tc.tile_pool(name="ct", bufs=4) as ctp,
            tc.tile_pool(name="psS", bufs=4, space="PSUM") as psS,
            tc.tile_pool(name="psO", bufs=2, space="PSUM") as psO,
            tc.tile_pool(name="psSum", bufs=2, space="PSUM") as psSum,
        ):
            xg_sb = res2.tile([P, NTK, GF], F32R)  # x[:, gsl] chunked by tk
            for i in range(NTK):
                nc.sync.dma_start(
                    out=xg_sb[:, i, :], in_=xg[i * P : (i + 1) * P, :]
                )
            oT_sb = res2.tile([P, GH, T], F32R)   # attention out, transposed
            wo_sb = res2.tile([P, GH, D], F32R)   # Wo[:, gsl].T chunked
            wor = woT.rearrange("(m p) d -> p m d", p=P)
            for m in range(GH):
                nc.sync.dma_start(out=wo_sb[:, m, :], in_=wor[:, m, :])

            pending = None
            for j in range(NTQ):
                qsl = slice(j * TQ, (j + 1) * TQ)
                nkk = (j + 1) * (TQ // P)  # causal: tk chunks needed
                # B: attention for each head on this query chunk
                for h in range(GH):
                    ps_o = psO.tile([P, TQ], F32, name="ps_o")
                    ps_sum = psSum.tile([1, TQ], F32, name="ps_sum")
                    wt_prev = None
                    for i in range(nkk):
                        ps_s = psS.tile([P, TQ], F32, name="ps_s", tag="ps_s")
                        nc.tensor.matmul(
                            ps_s,
                            kT_sb[:, h, i * P : (i + 1) * P],
                            qT_sb[:, h, qsl],
                            start=True,
                            stop=True,
                        )
                        wt = wtp.tile([P, TQ], F32R, name="wt")
                        nc.scalar.activation(
                            wt, ps_s, mybir.ActivationFunctionType.Exp,
                            scale=rsqrt_hd,
                        )
                        g = i - (TQ // P) * j
                        if g >= 0:  # diagonal tile: zero where tk > tq
                            nc.gpsimd.affine_select(
                                out=wt,
                                in_=wt,
                                pattern=[[1, TQ]],
                                compare_op=mybir.AluOpType.is_ge,
                                fill=0.0,
                                base=-(P * g),
                                channel_multiplier=-1,
                            )
                        nc.tensor.matmul(
                            ps_o,
                            xg_sb[:, i, h * HD : (h + 1) * HD],
                            wt,
                            start=(i == 0), stop=(i == nkk - 1),
                        )
                        # colsum: DVE pair-sums halve the PE's ones-matmuls
                        if i % 2 == 1:
                            wpair = prp.tile([P, TQ], F32R, name="wpair")
                            nc.vector.tensor_add(wpair, wt_prev, wt)
                            nc.tensor.matmul(
                                ps_sum, ones_sb[:, 0:1], wpair,
                                start=(i == 1), stop=(i == nkk - 1),
                            )
                        wt_prev = wt
                    # normalization (1/colsum -> partition broadcast ->
                    # (ps_o*c)*recip) is deferred one head so neither the
                    # gpsimd queue nor the PE ever waits on the chain
                    if pending is not None:
                        _emit_normalize(nc, smp, wtp, oT_sb, cT_sb, mult,
                                        *pending)
                    pending = (h, j, ps_o, ps_sum)
                # C: output projection, deferred one chunk so the PE
                # has B(j) queued while C(j-1)'s oT dependencies settle
                if j > 0:
                    _emit_outproj(nc, psS, ctp, wo_sb, oT_sb, out, j - 1)
            _emit_normalize(nc, smp, wtp, oT_sb, cT_sb, mult, *pending)
            _emit_outproj(nc, psS, ctp, wo_sb, oT_sb, out, NTQ - 1)


def _emit_normalize(nc, smp, wtp, oT_sb, cT_sb, mult, h, j, ps_o, ps_sum):
    """1/colsum on one partition, gpsimd partition broadcast, then
    (ps_o * c[p]) * recip in one DVE pass."""
    qsl = slice(j * TQ, (j + 1) * TQ)
    recip = smp.tile([1, TQ], F32, name="recip")
    nc.vector.reciprocal_approx_fast(out=recip, in_=ps_sum)
    rb = wtp.tile([P, TQ], F32, name="rb")
    nc.gpsimd.partition_broadcast(rb, recip)
    nc.vector.scalar_tensor_tensor(
        out=oT_sb[:, h, qsl],
        in0=ps_o,
        scalar=cT_sb[:, h : h + 1],
        in1=rb,
        op0=mult,
        op1=mult,
    )


def _emit_outproj(nc, psS, ctp, wo_sb, oT_sb, out, j):
    qsl = slice(j * TQ, (j + 1) * TQ)
    for dk in range(DK):
        ps = psS.tile([P, TQ], F32, name="psC_t", tag="ps_s")
        for m in range(GH):
            nc.tensor.matmul(
                ps,
                wo_sb[:, m, dk * P : (dk + 1) * P],
                oT_sb[:, m, qsl],
                start=(m == 0),
                stop=(m == GH - 1),
            )
        ct = ctp.tile([P, TQ], F32, name="ct")
        nc.scalar.copy(ct, ps)
        nc.sync.dma_start(out=out[dk * P : (dk + 1) * P, qsl], in_=ct)


def build_program():
    nc = bacc.Bacc(
        "TRN2", target_bir_lowering=False, debug=False, num_devices=NCORES
    )
    f = F32
    xT = nc.dram_tensor("xT", [D, T], F32R, kind="ExternalInput").ap()
    xg = nc.dram_tensor("xg", [T, GF], F32R, kind="ExternalInput").ap()
    wqT = nc.dram_tensor("wqT", [D, GF], F32R, kind="ExternalInput").ap()
    wkT = nc.dram_tensor("wkT", [D, GF], F32R, kind="ExternalInput").ap()
    woT = nc.dram_tensor("woT", [GF, D], F32R, kind="ExternalInput").ap()
    cT = nc.dram_tensor("cT", [HD, GH], f, kind="ExternalInput").ap()
    bqT = nc.dram_tensor("bqT", [HD, GH], f, kind="ExternalInput").ap()
    bkT = nc.dram_tensor("bkT", [HD, GH], f, kind="ExternalInput").ap()
    ones = nc.dram_tensor("ones", [P, P], F32R, kind="ExternalInput").ap()
    out = nc.dram_tensor("out", [D, T], f, kind="ExternalOutput").ap()

    with tile.TileContext(nc) as tc:
        _body(tc, xT, xg, wqT, wkT, woT, cT, bqT, bkT, ones, out)
    nc.compile()
    return nc


def _causal_masks() -> np.ndarray:
    """mask[g][p, f] = 1 iff tk <= tq for boundary tile offset g*128."""
    p = np.arange(P)[:, None]
    f = np.arange(TQ)[None, :]
    return np.stack(
        [(f >= p + g * P).astype(np.float32) for g in range(G)], axis=0
    )


_NC_CACHE = None
LAST_RESULT = None
TRACE = False


def kernel(x, Wq, bq, Wk, bk, Wvq, bvq, v_keys, v_embed, Wo, bo):
    global _NC_CACHE, LAST_RESULT
    x = np.asarray(x, np.float32)
    Wq = np.asarray(Wq, np.float32)
    bq = np.asarray(bq, np.float32)
    Wk = np.asarray(Wk, np.float32)
    bk = np.asarray(bk, np.float32)
    v_embed = np.asarray(v_embed, np.float32)
    Wo = np.asarray(Wo, np.float32)
    bo = np.asarray(bo, np.float32)

    c = 2.0 * v_embed[:G].sum(axis=0)
    in_maps = []
    for core in range(NCORES):
        b, g = divmod(core, G)
        gsl = slice(g * GF, (g + 1) * GF)
        in_maps.append(
            {
                "xT": np.ascontiguousarray(x[b].T),
                "xg": np.ascontiguousarray(x[b][:, gsl]),
                "wqT": np.ascontiguousarray(Wq[gsl, :].T),
                "wkT": np.ascontiguousarray(Wk[gsl, :].T),
                "woT": np.ascontiguousarray(Wo[:, gsl].T),
                "cT": np.ascontiguousarray(c[gsl].reshape(GH, HD).T),
                "bqT": np.ascontiguousarray(bq[gsl].reshape(GH, HD).T),
                "bkT": np.ascontiguousarray(bk[gsl].reshape(GH, HD).T),
                "ones": np.ones((P, P), np.float32),
            }
        )

    if _NC_CACHE is None:
        _NC_CACHE = build_program()
    res = run_bass_kernel_spmd(
        _NC_CACHE, in_maps, list(range(NCORES)), trace=TRACE
    )
    LAST_RESULT = res

    out = np.zeros((B, T, D), np.float32)
    for core in range(NCORES):
        b = core // G
        out[b] += res.results[core]["out"].T
    out += bo[None, None, :]
    return out


if __name__ == "__main__":
    nc = build_program()
    print("built ok")



# revision 7
# speedup vs baseline: 1.0304x; 1.0304x over previous
"""Trainium2 Bass kernel for nn_DMHA_3255585210402 (retrieval_knn DMHA).

Key algebraic fact: TOPK == NVK == 4, so jax.lax.top_k over the size-4 v_keys
axis selects *all* entries; the gather+sum over (DVH, TOPK) reduces to a
constant vector c = 2 * v_embed[0:4].sum(0) and compute_value_states collapses
to  v = x * c  (c is folded into the xg input host-side, so V is free).

The module is causal MHA (B=2, H=16, T=2048, HD=128, D=2048) with
elementwise-scaled V.  Sharding: 8 cores = 2 batches x 4 head-groups; each
core does qT/kT projections, attention for its 4 heads, and the partial
output projection Wo[:, gsl].T @ oT; host sums 4 partials per batch + bo.

Schedule (PE-bound, ~250us of matmul columns at 2.4GHz):
 - everything bf16 into the PE (1 cyc/row, same as f32r, but half the DMA
   and SBUF, and 2-4x DVE modes); PSUM/bias/out stay fp32.
 - phase A per-(proj,head)-sequential over dk so only ~2 PSUM banks are
   live -> one shared top-level PSUM pool, no pool-swap stall into B.
   Per-head weight DMA layout so the first matmul starts ~2us in.
 - phase B per (j,h): QK emitted L=3 ahead of PV so the PE rides through
   the QK->exp->PV chain; causal masking is a DVE bf16 multiply with
   precomputed mask tiles (gpsimd only does the recip row-broadcast).
 - softmax colsums: 4-way DVE stt-add tree (4x perf mode) then a ones-
   matmul per quad, deferred into the next head block behind C fillers.
 - phase C (out proj) interleaved 4 dk-groups per head block one j behind;
   PSUM->SBUF copies on DVE (Act only runs exps + phase-A bias copies).
"""

import math

import numpy as np
import ml_dtypes

import concourse.bass as bass
import concourse.mybir as mybir
import concourse.tile as tile
from concourse import bacc
from concourse.bass_utils import run_bass_kernel_spmd

B, T, D = 2, 2048, 2048
H, HD = 16, 128
G = 4              # head-groups (cores per batch)
GH = H // G        # heads per core
GF = GH * HD       # projected features per core (512)
NCORES = 8
P = 128            # partitions
TQ = 512           # tq chunk width (psum bank / fp32 moving max)
F32 = mybir.dt.float32
BF16 = mybir.dt.bfloat16

DK = D // P        # 16 contraction chunks for projections
NTQ = T // TQ      # 4 query chunks
NTK = T // P       # 16 key chunks

MULT = mybir.AluOpType.mult
ADD = mybir.AluOpType.add


def _emit_c_group(nc, psB, ctp, wo_sb, oT_sb, out, j, dk):
    """One output-projection dk-group: accumulate 4 heads, DVE copy, store."""
    qsl = slice(j * TQ, (j + 1) * TQ)
    ps = psB.tile([P, TQ], F32, name="psC", tag="psb")
    for m in range(GH):
        nc.tensor.matmul(
            ps,
            wo_sb[:, m, dk * P : (dk + 1) * P],
            oT_sb[:, m, qsl],
            start=(m == 0),
            stop=(m == GH - 1),
        )
    ct = ctp.tile([P, TQ], F32, name="ct")
    nc.vector.tensor_copy(ct, ps)
    nc.sync.dma_start(out=out[dk * P : (dk + 1) * P, qsl], in_=ct)


def _body(tc, xT, xg, wqh, wkh, woT, bqT, bkT, ones, masks, out):
    nc = tc.nc
    rsqrt_hd = 1.0 / math.sqrt(HD)

    with (
        tc.tile_pool(name="const", bufs=1) as const,
        tc.tile_pool(name="res", bufs=1) as res,
        tc.tile_pool(name="psB", bufs=5, space="PSUM") as psB,
        tc.tile_pool(name="psO", bufs=2, space="PSUM") as psO,
        tc.tile_pool(name="psSum", bufs=1, space="PSUM") as psSum,
        tc.tile_pool(name="wt", bufs=9) as wtp,
        tc.tile_pool(name="quad", bufs=6) as qdp,
        tc.tile_pool(name="rb", bufs=2) as rbp,
        tc.tile_pool(name="recip", bufs=2) as rcp,
        tc.tile_pool(name="ct", bufs=4) as ctp,
    ):
        qT_sb = res.tile([P, GH, T], BF16)    # q, transposed per head
        kT_sb = res.tile([P, GH, T], BF16)
        oT_sb = res.tile([P, GH, T], BF16)    # attention out (unnormalized->normalized)
        wq_sb = res.tile([P, GH, DK * HD], BF16)
        wk_sb = res.tile([P, GH, DK * HD], BF16)
        xg_sb = res.tile([P, NTK, GF], BF16)  # c-scaled x[:, gsl] chunked by tk
        wo_sb = res.tile([P, GH, D], BF16)    # Wo[:, gsl].T chunked by head
        mask_sb = const.tile([P, G, TQ], BF16)
        ones_sb = const.tile([P, 1], BF16)
        bq_sb = const.tile([HD, GH], F32)
        bk_sb = const.tile([HD, GH], F32)

        wqr = wqh.rearrange("(h p) f -> h p f", h=GH)
        wkr = wkh.rearrange("(h p) f -> h p f", h=GH)
        wor = woT.rearrange("(m p) d -> p m d", p=P)
        maskr = masks.rearrange("(g p) f -> g p f", g=G)

        # --- phase A: q/k projections, per-(proj,head)-sequential over dk ---
        with tc.tile_pool(name="xt", bufs=30) as xtp:
            # DMA order: first head's weights, then tci=0 x chunks, then the
            # rest of the weights, small constants, gpsimd library.
            nc.sync.dma_start(out=wq_sb[:, 0, :], in_=wqr[0, :, :])
            nc.sync.dma_start(out=wk_sb[:, 0, :], in_=wkr[0, :, :])
            xts = [None] * DK
            for dk in range(DK):
                xt0 = xtp.tile([P, TQ], BF16, name="xt")
                nc.sync.dma_start(out=xt0, in_=xT[dk * P : (dk + 1) * P, 0:TQ])
                xts[dk] = xt0
            for h in range(1, GH):
                nc.sync.dma_start(out=wq_sb[:, h, :], in_=wqr[h, :, :])
                nc.sync.dma_start(out=wk_sb[:, h, :], in_=wkr[h, :, :])
            nc.sync.dma_start(out=ones_sb, in_=ones)
            nc.sync.dma_start(out=bq_sb, in_=bqT)
            nc.sync.dma_start(out=bk_sb, in_=bkT)
            for g in range(G):
                nc.sync.dma_start(out=mask_sb[:, g, :], in_=maskr[g, :, :])
            from concourse import library_config
            nc.gpsimd.load_library(library_config.attn)

            for tci in range(NTQ):
                tsl = slice(tci * TQ, (tci + 1) * TQ)
                # prefetch next tci's x chunks (pool is 2x deep)
                if tci + 1 < NTQ:
                    nsl = slice((tci + 1) * TQ, (tci + 2) * TQ)
                    nxts = [None] * DK
                    for dk in range(DK):
                        xt = xtp.tile([P, TQ], BF16, name="xt")
                        nc.sync.dma_start(
                            out=xt, in_=xT[dk * P : (dk + 1) * P, nsl]
                        )
                        nxts[dk] = xt
                if tci == 1:
                    # B/C-phase inputs, off the critical DMA path
                    for i in range(NTK):
                        nc.sync.dma_start(
                            out=xg_sb[:, i, :], in_=xg[i * P : (i + 1) * P, :]
                        )
                    for m in range(GH):
                        nc.sync.dma_start(out=wo_sb[:, m, :], in_=wor[:, m, :])
                for w_sb, dstT, bias in (
                    (wq_sb, qT_sb, bq_sb),
                    (wk_sb, kT_sb, bk_sb),
                ):
                    for h in range(GH):
                        ps = psB.tile([P, TQ], F32, name="psA", tag="psb")
                        for dk in range(DK):
                            nc.tensor.matmul(
                                ps,
                                w_sb[:, h, dk * HD : (dk + 1) * HD],
                                xts[dk],
                                start=(dk == 0),
                                stop=(dk == DK - 1),
                            )
                        nc.scalar.activation(
                            dstT[:, h, tsl],
                            ps,
                            mybir.ActivationFunctionType.Identity,
                            bias=bias[:, h : h + 1],
                        )
                if tci + 1 < NTQ:
                    xts = nxts

        # --- phases B+C: attention + out-proj, interleaved per head block ---
        # Per head block the i-loop runs QK with L-deep PV lookahead; quad
        # colsum matmuls are emitted 8 iterations behind their QK (so the
        # exp->mask->add chain has settled), remaining quads plus the
        # recip/broadcast/normalize finalize at the start of the NEXT block.
        # C groups for chunk j-1 fill blocks h1..h3 of chunk j (after the
        # flush at h0 has emitted the normalize for (j-1, h3)).
        state = None
        cfill = []  # (j, dk) out-proj groups awaiting emission

        def emit_quad_mm(st):
            if st["ps_sum"] is None:
                st["ps_sum"] = psSum.tile([1, TQ], F32, name="ps_sum")
            qi = st["n_summed"]
            nc.tensor.matmul(
                st["ps_sum"], ones_sb, st["quads"][qi],
                start=(qi == 0), stop=(qi == st["nq"] - 1),
            )
            st["n_summed"] += 1

        def finalize(st):
            while st["n_summed"] < st["nq"]:
                emit_quad_mm(st)
            qsl = slice(st["j"] * TQ, (st["j"] + 1) * TQ)
            recip = rcp.tile([1, TQ], F32, name="recip")
            nc.vector.reciprocal_approx_fast(out=recip, in_=st["ps_sum"])
            rb = rbp.tile([P, TQ], F32, name="rb")
            nc.gpsimd.partition_broadcast(rb, recip)
            nc.vector.tensor_mul(oT_sb[:, st["h"], qsl], st["ps_o"], rb)

        for j in range(NTQ):
            qsl = slice(j * TQ, (j + 1) * TQ)
            nkk = (j + 1) * (TQ // P)
            L = 4 if j == 0 else 3
            for h in range(GH):
                n_c = min((0, 6, 5, 5)[h], len(cfill))
                for (cj, cdk) in cfill[:n_c]:
                    _emit_c_group(nc, psB, ctp, wo_sb, oT_sb, out, cj, cdk)
                del cfill[:n_c]
                if state is not None:
                    finalize(state)
                st = {
                    "h": h, "j": j, "nq": nkk // 4, "n_summed": 0,
                    "ps_sum": None, "quads": [],
                    "ps_o": psO.tile([P, TQ], F32, name="ps_o"),
                }
                wts = [None] * nkk
                wpair = None
                for idx in range(nkk + L):
                    if idx < nkk:
                        ps_s = psB.tile([P, TQ], F32, name="ps_s", tag="psb")
                        nc.tensor.matmul(
                            ps_s,
                            kT_sb[:, h, idx * P : (idx + 1) * P],
                            qT_sb[:, h, qsl],
                            start=True,
                            stop=True,
                        )
                        wt = wtp.tile([P, TQ], BF16, name="wt")
                        nc.scalar.activation(
                            wt, ps_s, mybir.ActivationFunctionType.Exp,
                            scale=rsqrt_hd,
                        )
                        g = idx - (TQ // P) * j
                        if g >= 0:  # diagonal tile: zero where tk > tq
                            nc.vector.tensor_mul(wt, wt, mask_sb[:, g, :])
                        wts[idx] = wt
                        if idx % 2 == 1:
                            wp = wtp.tile([P, TQ], BF16, name="wp")
                            nc.vector.scalar_tensor_tensor(
                                out=wp, in0=wts[idx - 1], scalar=1.0,
                                in1=wt, op0=MULT, op1=ADD,
                            )
                            if idx % 4 == 3:
                                wq_t = qdp.tile([P, TQ], BF16, name="wq4")
                                nc.vector.scalar_tensor_tensor(
                                    out=wq_t, in0=wpair, scalar=1.0,
                                    in1=wp, op0=MULT, op1=ADD,
                                )
                                st["quads"].append(wq_t)
                            wpair = wp
                    if idx >= 8 and idx % 4 == 0 and (idx - 8) // 4 < st["nq"]:
                        emit_quad_mm(st)
                    if idx >= L:
                        i = idx - L
                        nc.tensor.matmul(
                            st["ps_o"],
                            xg_sb[:, i, h * HD : (h + 1) * HD],
                            wts[i],
                            start=(i == 0),
                            stop=(i == nkk - 1),
                        )
                state = st
                if h == GH - 1:
                    cfill.extend((j, dk) for dk in range(DK))
        finalize(state)
        for (cj, cdk) in cfill:
            _emit_c_group(nc, psB, ctp, wo_sb, oT_sb, out, cj, cdk)


def build_program():
    nc = bacc.Bacc(
        "TRN2", target_bir_lowering=False, debug=False, num_devices=NCORES
    )
    xT = nc.dram_tensor("xT", [D, T], BF16, kind="ExternalInput").ap()
    xg = nc.dram_tensor("xg", [T, GF], BF16, kind="ExternalInput").ap()
    wqh = nc.dram_tensor("wqh", [GH * P, DK * HD], BF16, kind="ExternalInput").ap()
    wkh = nc.dram_tensor("wkh", [GH * P, DK * HD], BF16, kind="ExternalInput").ap()
    woT = nc.dram_tensor("woT", [GF, D], BF16, kind="ExternalInput").ap()
    bqT = nc.dram_tensor("bqT", [HD, GH], F32, kind="ExternalInput").ap()
    bkT = nc.dram_tensor("bkT", [HD, GH], F32, kind="ExternalInput").ap()
    ones = nc.dram_tensor("ones", [P, 1], BF16, kind="ExternalInput").ap()
    masks = nc.dram_tensor("masks", [G * P, TQ], BF16, kind="ExternalInput").ap()
    out = nc.dram_tensor("out", [D, T], F32, kind="ExternalOutput").ap()

    with tile.TileContext(nc) as tc:
        _body(tc, xT, xg, wqh, wkh, woT, bqT, bkT, ones, masks, out)
    nc.compile()
    return nc


def _causal_masks() -> np.ndarray:
    """mask[g][p, f] = 1 iff tk <= tq for boundary tile offset g*128."""
    p = np.arange(P)[:, None]
    f = np.arange(TQ)[None, :]
    return np.stack(
        [(f >= p + g * P).astype(np.float32) for g in range(G)], axis=0
    )


def _whead(W: np.ndarray) -> np.ndarray:
    """[GF, D] weight slice -> per-head DMA layout [GH*P, DK*HD] (bf16)."""
    return np.ascontiguousarray(
        W.reshape(GH, HD, DK, P).transpose(0, 3, 2, 1).reshape(GH * P, DK * HD)
    )


_NC_CACHE = None
LAST_RESULT = None
TRACE = False


def kernel(x, Wq, bq, Wk, bk, Wvq, bvq, v_keys, v_embed, Wo, bo):
    global _NC_CACHE, LAST_RESULT
    bf = ml_dtypes.bfloat16
    x = np.asarray(x, np.float32)
    Wq = np.asarray(Wq, np.float32)
    bq = np.asarray(bq, np.float32)
    Wk = np.asarray(Wk, np.float32)
    bk = np.asarray(bk, np.float32)
    v_embed = np.asarray(v_embed, np.float32)
    Wo = np.asarray(Wo, np.float32)
    bo = np.asarray(bo, np.float32)

    c = 2.0 * v_embed[:G].sum(axis=0)
    masks = np.ascontiguousarray(_causal_masks().reshape(G * P, TQ))
    in_maps = []
    for core in range(NCORES):
        b, g = divmod(core, G)
        gsl = slice(g * GF, (g + 1) * GF)
        in_maps.append(
            {
                "xT": np.ascontiguousarray(x[b].T).astype(bf),
                "xg": np.ascontiguousarray(x[b][:, gsl] * c[gsl]).astype(bf),
                "wqh": _whead(Wq[gsl, :]).astype(bf),
                "wkh": _whead(Wk[gsl, :]).astype(bf),
                "woT": np.ascontiguousarray(Wo[:, gsl].T).astype(bf),
                "bqT": np.ascontiguousarray(bq[gsl].reshape(GH, HD).T),
                "bkT": np.ascontiguousarray(bk[gsl].reshape(GH, HD).T),
                "ones": np.ones((P, 1), bf),
                "masks": masks.astype(bf),
            }
        )

    if _NC_CACHE is None:
        _NC_CACHE = build_program()
    res = run_bass_kernel_spmd(
        _NC_CACHE, in_maps, list(range(NCORES)), trace=TRACE
    )
    LAST_RESULT = res

    out = np.zeros((B, T, D), np.float32)
    for core in range(NCORES):
        b = core // G
        out[b] += res.results[core]["out"].T
    out += bo[None, None, :]
    return out


if __name__ == "__main__":
    nc = build_program()
    print("built ok")
